# revision 30
# baseline (speedup 1.0000x reference)
"""Trainium2 Bass kernel for GNN mean aggregation (nn_AggrGSMean).

Computes, for t in {0,1}:
    out_t[b, v, :] = segment_sum(features_t over edges with dest v) / degree[b, v, t]
where degree[b, v, t] = max(count(adjacency[b, v, t, :] >= 0), 1).

Strategy (graph-partition sharding per the problem's sharding hint):
- Host: partition edges by destination-vertex range across 8 cores, sort each
  core's edges by destination, and fold 1/degree into the features (f32
  multiply, then bf16).  Same-destination edges are PAIRED (Q=2, odd counts
  padded with a zero edge); a pair-row carries both edges' features in (q f)
  word order.  Vertices are bin-packed into 128-vertex blocks whose pair
  counts just fit 8 or 9 full 128-row tiles (serpentine deal by degree +
  swap repair), so the shared static per-slot profile (max over cores and
  tables at each sorted rank) wastes <2%.  Adjacent slots interleave per
  partition so one DMA loads a slot pair.  Destination slot-vertex ids ship
  as a separate bf16 "vin" stream [128, total_tiles].
- Device (per core): per slot-pair and table: one feature DMA (alternating
  the SP / Activation / GPSIMD DMA rings for aggregate bandwidth); one-hots
  [128 pair-rows x 128 vslots] built on DVE by iota==vin in a transposed
  (v i) all-packed layout batched 8 tiles per instruction (odd 9th tiles go
  to ScalarE via relu(1-(iota-v)^2) so DVE chunks keep even lhsT strides);
  per tile one matmul accumulates onehot.T @ pairs into a quarter of a
  shared PSUM tile [128, 4*(q f)] f32 (one full bank per slot-quad); one DVE
  reduce per same-group slot run adds the q halves straight into the bf16
  group output tile, DMA'd out per 7-slot group.
"""

import sys

if "/opt/trn_rl_repo" not in sys.path:
    sys.path.insert(0, "/opt/trn_rl_repo")

import ml_dtypes
import numpy as np

# Problem constants (hardcoded per contract)
B, V, T, N, F, M = 1, 100000, 2, 32, 64, 1600000
NCORES = 8
BLK = 128           # pair-rows per tile (matmul contraction)
BLK_V = 128         # vertices per block / one-hot width
Q = 2               # edges pre-summed per pair-row
ROW_W = Q * F       # bf16 words per pair-row (128)
G = 7               # slots per output group
VLOC = V // NCORES          # 12500
NBLK = -(-VLOC // BLK_V)    # 98
NGRP = -(-NBLK // G)        # 14
VPAD = NBLK * BLK_V         # 12544


class Cfg:
    def __init__(self):
        self.V = V
        self.NCORES = NCORES
        self.VLOC = VLOC
        self.NBLK = NBLK
        self.VPAD = VPAD


_DEFAULT_CFG = Cfg()
_NC_CACHE = {}


def build_device_program(
    profile, cfg=_DEFAULT_CFG, act_frac=0.06, oh_chunk=8, gp_mod=8, act_period=13
):
    """Build + compile the per-core Bass program.

    One-hot builds run on DVE in the transposed (v i) all-packed layout,
    batched oh_chunk tiles per instruction (stride-8 lhsT stays 16B-aligned
    for LDWEIGHTS); the odd 9th tile of 9-tile slots goes to ScalarE via
    relu(1-(iota-v)^2).  Feature DMAs alternate the SP and Activation HWDGE
    rings with every gp_mod'th on the GPSIMD SWDGE ring."""
    from contextlib import ExitStack

    import concourse.tile as tile
    from concourse import bacc, mybir

    f32 = mybir.dt.float32
    bf16 = mybir.dt.bfloat16
    assert len(profile) == NBLK
    prof = np.asarray(profile, dtype=np.int64)
    t_max = int(prof.max())
    TT = int(prof.sum())                      # total tiles per table
    # slot-PAIR interleaved stream: each partition's two slot segments are
    # contiguous, so one 2D DMA covers a pair of any widths
    wpair = (prof[0::2] + prof[1::2]) * ROW_W          # words per partition
    peb = np.concatenate([[0], np.cumsum(BLK * wpair)]).astype(np.int64)
    tb = np.concatenate([[0], np.cumsum(prof)]).astype(np.int64)
    total_elems = int(peb[-1])

    nc = bacc.Bacc("TRN2", target_bir_lowering=False, debug=False)
    feat_d = [
        nc.dram_tensor(f"feat{t}", [total_elems], bf16, kind="ExternalInput").ap()
        for t in range(T)
    ]
    vin_d = [
        nc.dram_tensor(f"vin{t}", [BLK, TT], bf16, kind="ExternalInput").ap()
        for t in range(T)
    ]
    iota_d = nc.dram_tensor("iota", [BLK, BLK_V], bf16, kind="ExternalInput").ap()
    iotar_d = nc.dram_tensor(
        "iotar", [BLK, oh_chunk * BLK_V], bf16, kind="ExternalInput"
    ).ap()
    out_d = nc.dram_tensor(
        "out", [NGRP, BLK_V, G * T * F], bf16, kind="ExternalOutput"
    ).ap()

    with tile.TileContext(nc) as tc, ExitStack() as ctx:
        const = ctx.enter_context(tc.tile_pool(name="const", bufs=1))
        featp = ctx.enter_context(tc.tile_pool(name="featp", bufs=8))
        ohdp = ctx.enter_context(tc.tile_pool(name="ohdp", bufs=6))
        ohap = ctx.enter_context(tc.tile_pool(name="ohap", bufs=7))
        outp = ctx.enter_context(tc.tile_pool(name="outp", bufs=3))
        psump = ctx.enter_context(tc.tile_pool(name="psum", bufs=8, space="PSUM"))

        # Const loads ride the Activation engine's HWDGE queues so they can
        # never get stuck behind feature DMAs (sync queues) that wait on
        # tile-pool recycling.
        iota_t = const.tile([BLK, BLK_V], bf16)
        nc.scalar.dma_start(out=iota_t[:], in_=iota_d[:])
        iotar_t = const.tile([BLK, oh_chunk * BLK_V], bf16)
        nc.scalar.dma_start(out=iotar_t[:], in_=iotar_d[:])
        vin_t = []
        vinf_t = []
        for t in range(T):
            vt = const.tile([BLK, TT], bf16, tag=f"vin{t}")
            # 4-chunk column split so the load spreads across DMA queues
            bnds = [TT * i // 4 for i in range(5)]
            for a, b in zip(bnds[:-1], bnds[1:]):
                if b > a:
                    nc.gpsimd.dma_start(out=vt[:, a:b], in_=vin_d[t][:, a:b])
            vin_t.append(vt)
            # f32 negated copy for the ScalarE activation-bias one-hot path
            vf = const.tile([BLK, TT], f32, tag=f"vinf{t}")
            nc.vector.tensor_scalar(
                vf[:], vt[:], -1.0, None, op0=mybir.AluOpType.mult
            )
            vinf_t.append(vf)

        # --- one-hot build planning: odd slots give their last tile to
        # ScalarE so every DVE chunk keeps an even lhsT stride (aligned
        # LDWEIGHTS); remaining tiles batch in chunks of oh_chunk.  Every
        # act_period'th slot-table goes entirely to ScalarE to offload DVE.
        slot_ctr = [0]

        def plan_chunks(t_s):
            slot_ctr[0] += 1
            chunks = []
            end = t_s
            if t_s % 2 == 1 and t_s > 1:
                chunks.append(("act", t_s - 1, 1))
                end = t_s - 1
            i = 0
            while i < end:
                k = min(oh_chunk, end - i)
                chunks.append(("b", i, k))
                i += k
            return chunks

        ndma = [0]
        out_tiles = {}

        def compute_slot(s, t, feat_t, off, ps, half):
            """One (slot, table) pass: one-hots + matmuls into psum half."""
            t_s = int(prof[s])
            g, so = divmod(s, G)
            if g not in out_tiles:
                ot = outp.tile([BLK_V, G * T * F], bf16, tag="outg")
                out_tiles[g] = ot
            refs = [None] * t_s
            for kind, i0, k in plan_chunks(t_s):
                c0 = int(tb[s]) + i0
                if kind == "act":
                    bias = vinf_t[t][:, c0 : c0 + 1]
                    y = ohap.tile([BLK, BLK_V], bf16, tag="y")
                    nc.scalar.activation(
                        y[:], iota_t[:],
                        mybir.ActivationFunctionType.Square,
                        bias=bias, scale=1.0,
                    )
                    oh = ohap.tile([BLK, BLK_V], bf16, tag="oha")
                    nc.scalar.activation(
                        oh[:], y[:], mybir.ActivationFunctionType.Relu,
                        bias=1.0, scale=-1.0,
                    )
                    refs[i0] = (oh, 0, 1)
                else:
                    oh = ohdp.tile([BLK, oh_chunk * BLK_V], bf16, tag="ohb")
                    oh3 = oh[:, : k * BLK_V].rearrange("p (v i) -> p v i", i=k)
                    in0 = iotar_t[:].rearrange(
                        "p (v i) -> p v i", i=oh_chunk
                    )[:, :, :k]
                    in1 = (
                        vin_t[t][:, c0 : c0 + k]
                        .unsqueeze(1)
                        .broadcast_to([BLK, BLK_V, k])
                    )
                    nc.vector.tensor_tensor(
                        oh3, in0, in1, op=mybir.AluOpType.is_equal
                    )
                    for j in range(k):
                        refs[i0 + j] = (oh, j, k)
            for i in range(t_s):
                oh, idx, stride = refs[i]
                if stride == 1:
                    lhsT = oh[:, idx : idx + BLK_V]
                else:
                    lhsT = oh[:, : stride * BLK_V].rearrange(
                        "p (v i) -> p i v", i=stride
                    )[:, idx, :]
                nc.tensor.matmul(
                    ps[:, half * ROW_W : (half + 1) * ROW_W],
                    lhsT=lhsT,
                    rhs=feat_t[:, (off + i) * ROW_W : (off + i + 1) * ROW_W],
                    start=(i == 0),
                    stop=(i == t_s - 1),
                )

        for q0 in range(0, NBLK, 4):
            qn = min(4, NBLK - q0)
            for t in range(T):
                fts = []
                for pp in range(0, qn, 2):
                    pid = (q0 + pp) // 2
                    w2 = int(wpair[pid])
                    feat_t = featp.tile(
                        [BLK, 2 * t_max * ROW_W], bf16, tag="feat"
                    )
                    srcap = feat_d[t][
                        int(peb[pid]) : int(peb[pid + 1])
                    ].rearrange("(e w) -> e w", w=w2)
                    ndma[0] += 1
                    if ndma[0] % gp_mod == 0:
                        deng = nc.gpsimd
                    elif ndma[0] % 2 == 0:
                        deng = nc.scalar
                    else:
                        deng = nc.sync
                    deng.dma_start(out=feat_t[:, :w2], in_=srcap)
                    fts.append(feat_t)
                ps = psump.tile([BLK_V, 4 * ROW_W], f32)
                for pp in range(0, qn, 2):
                    s0 = q0 + pp
                    compute_slot(s0, t, fts[pp // 2], 0, ps, pp)
                    compute_slot(
                        s0 + 1, t, fts[pp // 2], int(prof[s0]), ps, pp + 1
                    )
                # reduce per same-group slot run within the quad
                with nc.allow_low_precision(reason="bf16 mean output"):
                    s = q0
                    while s < q0 + qn:
                        g = s // G
                        e = min(q0 + qn, (g + 1) * G)
                        L = e - s
                        o = s - q0
                        so = s % G
                        vv = out_tiles[g][:].rearrange(
                            "p (so tt f) -> p so tt f", so=G, tt=T
                        )
                        nc.vector.tensor_reduce(
                            vv[:, so : so + L, t, :],
                            ps[
                                :, o * ROW_W : (o + L) * ROW_W
                            ].rearrange(
                                "p (sL q f) -> p sL f q", sL=L, q=Q
                            ),
                            axis=mybir.AxisListType.X,
                            op=mybir.AluOpType.add,
                        )
                        s = e
            for s in range(q0, q0 + qn):
                if s % G == G - 1:
                    g = s // G
                    nc.sync.dma_start(out=out_d[g], in_=out_tiles.pop(g)[:])

    nc.compile()
    return nc


def _pack_core(pv, x_high):
    """Pack VPAD vertices (pair counts pv) into NBLK bins of exactly BLK_V.

    Serpentine-deal by descending pv (near-equal sums), then swap-repair so
    at most x_high bins exceed 8 tiles (cap 9).  Returns bins [NBLK, BLK_V]
    of vertex ids."""
    order = np.argsort(-pv, kind="stable")
    deal = order.reshape(BLK_V, NBLK).copy()
    deal[1::2] = deal[1::2, ::-1]
    bins = np.ascontiguousarray(deal.T)          # [NBLK, BLK_V]
    sums = pv[bins].sum(axis=1)
    lo_cap, hi_cap = 8 * BLK, 9 * BLK
    hi = set(np.argsort(-sums)[:x_high])
    room = {h: hi_cap - sums[h] for h in hi}
    for b in range(NBLK):
        if b in hi:
            continue
        need = sums[b] - lo_cap
        it = 0
        while need > 0 and it < 40:
            it += 1
            h = max(room, key=room.get)
            if room[h] <= 0:
                break
            pb_ = pv[bins[b]]
            ph_ = pv[bins[h]]
            iu = int(np.argmax(pb_))
            iw = int(np.argmin(ph_))
            gain = int(pb_[iu] - ph_[iw])
            if gain <= 0:
                break
            gain = min(gain, int(room[h]) + 0)
            u, w = bins[b, iu], bins[h, iw]
            if pv[u] - pv[w] > room[h]:
                # find a better-matched u: largest pv[u] with delta <= room
                cand = np.where(pb_ - pv[w] <= room[h])[0]
                if len(cand) == 0:
                    break
                iu = int(cand[np.argmax(pb_[cand])])
                u = bins[b, iu]
                gain = int(pv[u] - pv[w])
                if gain <= 0:
                    break
            bins[b, iu], bins[h, iw] = w, u
            sums[b] -= gain
            sums[h] += gain
            room[h] -= gain
            need -= gain
    return bins


def shard_table(indices, x_high=None, cfg=_DEFAULT_CFG):
    """Per-edge placement: sort by destination, pair same-dest edges, and
    bin-pack vertices into blocks so slot tile counts are (mostly) 8 or 9."""
    v = np.ascontiguousarray(indices[:, 1]).astype(np.int64)
    order = np.argsort(v, kind="stable")
    vs = v[order]
    n_v = np.bincount(vs, minlength=V)
    starts = np.concatenate([[0], np.cumsum(n_v)])
    r = np.arange(len(vs), dtype=np.int64) - starts[vs]
    pv = (n_v + 1) // 2                                  # pairs per vertex
    core = vs // VLOC
    vloc_e = vs % VLOC

    pv_pad = np.zeros((NCORES, VPAD), dtype=np.int64)
    pv_pad[:, :VLOC] = pv.reshape(NCORES, VLOC)
    if x_high is None:
        tp = pv_pad.sum(axis=1)
        x_high = int(np.ceil((tp.max() - NBLK * 8 * BLK) / BLK)) + 1
        x_high = max(0, min(NBLK, x_high))

    vblk = np.empty((NCORES, VPAD), dtype=np.int64)
    vvin = np.empty((NCORES, VPAD), dtype=np.int64)
    pbb = np.empty((NCORES, VPAD), dtype=np.int64)   # pair base within block
    cnt = np.empty((NCORES, NBLK), dtype=np.int64)
    for c in range(NCORES):
        bins = _pack_core(pv_pad[c], x_high)
        flat = bins.reshape(-1)                      # slot index -> vertex
        slot_of = np.empty(VPAD, dtype=np.int64)
        slot_of[flat] = np.arange(VPAD)
        vblk[c] = slot_of // BLK_V
        vvin[c] = slot_of % BLK_V
        pv_by_slot = pv_pad[c][flat]
        cum = np.cumsum(pv_by_slot) - pv_by_slot     # exclusive
        blk_start = cum.reshape(NBLK, BLK_V)[:, 0]
        pb_by_slot = cum - np.repeat(blk_start, BLK_V)
        pbb[c] = pb_by_slot[slot_of]
        cnt[c] = pv_by_slot.reshape(NBLK, BLK_V).sum(axis=1)

    blk_e = vblk[core, vloc_e]
    vin_e = vvin[core, vloc_e]
    pr = pbb[core, vloc_e] + r // 2
    tiles = np.maximum(-(-cnt // BLK), 1)            # [NCORES, NBLK]
    return {
        "order": order, "core": core, "blk": blk_e, "vin": vin_e,
        "pr": pr, "q": (r & 1).astype(np.int64), "tiles": tiles,
        "vglob": vs, "vblk": vblk, "vvin": vvin,
    }


def make_profile(tables):
    """Shared slot tile profile + per (table, core) slot->block permutation.

    Returns (profile, meta) where meta carries the permutations and the
    per-table vertex->(block, vin) maps needed for output assembly."""
    perms = []
    sorted_tiles = []
    for tab in tables:
        perms_t = []
        for c in range(NCORES):
            tl = tab["tiles"][c]
            p = np.argsort(-tl, kind="stable")
            perms_t.append(p)
            sorted_tiles.append(tl[p])
        perms.append(np.stack(perms_t))
    profile = np.max(np.stack(sorted_tiles), axis=0)
    profile = np.maximum(profile, 1)
    meta = {
        "perm": perms,
        "vmap": [(tab["vblk"], tab["vvin"]) for tab in tables],
    }
    return [int(x) for x in profile], meta


def fill_streams(tab, features, rec_e, profile, perm, cfg=_DEFAULT_CFG):
    """Per-core bf16 feature stream (pre-scaled by 1/degree) + vin stream.

    Row layout is (q f): word q*F + f, so the PSUM pair halves are the
    contiguous column blocks [0:F] and [F:2F]."""
    prof = np.asarray(profile, dtype=np.int64)
    TT = int(prof.sum())
    wpair = (prof[0::2] + prof[1::2]) * ROW_W
    peb = np.concatenate([[0], np.cumsum(BLK * wpair)]).astype(np.int64)
    tb = np.concatenate([[0], np.cumsum(prof)]).astype(np.int64)
    TW = int(peb[-1])
    soff = np.zeros(NBLK, dtype=np.int64)
    soff[1::2] = prof[0::2] * ROW_W

    inv = np.empty((NCORES, NBLK), dtype=np.int64)
    for c in range(NCORES):
        inv[c, perm[c]] = np.arange(NBLK)

    scaled = features[tab["order"]] * rec_e[:, None]
    hi_u = scaled.astype(ml_dtypes.bfloat16).view(np.uint16)

    core = tab["core"]
    s_e = inv[core, tab["blk"]]
    p = tab["pr"] & (BLK - 1)
    i = tab["pr"] >> 7
    q = tab["q"]
    pid_e = s_e >> 1
    pos = (
        core * TW + peb[pid_e] + p * wpair[pid_e] + soff[s_e] + i * ROW_W + q * F
    )
    stream = np.zeros(NCORES * TW, dtype=np.uint16)
    cols = np.arange(F, dtype=np.int64)[None, :]
    stream[pos[:, None] + cols] = hi_u
    stream = stream.reshape(NCORES, TW).view(ml_dtypes.bfloat16)

    # vin stream [NCORES, 128, TT]; padding rows get -1 (never matches iota)
    vin_arr = np.full(NCORES * BLK * TT, -1.0, dtype=ml_dtypes.bfloat16)
    m0 = q == 0
    flat = core[m0] * (BLK * TT) + p[m0] * TT + (tb[s_e[m0]] + i[m0])
    vin_arr[flat] = tab["vin"][m0].astype(ml_dtypes.bfloat16)
    vin_arr = vin_arr.reshape(NCORES, BLK, TT)
    return stream, vin_arr


def edge_recip(adjacency, tab, t):
    """1/degree at each sorted edge's destination for table t."""
    adj = np.asarray(adjacency).reshape(V, T, N)
    deg = np.maximum((adj[:, t] >= 0).sum(axis=-1), 1).astype(np.float64)  # [V]
    rec = (1.0 / deg).astype(np.float32)
    return rec[tab["vglob"]]


def prepare_inputs(adjacency, indices0, features0, indices1, features1, cfg=_DEFAULT_CFG, oh_chunk=8):
    tab0 = shard_table(np.asarray(indices0), cfg=cfg)
    tab1 = shard_table(np.asarray(indices1), cfg=cfg)
    profile, meta = make_profile([tab0, tab1])

    r0 = edge_recip(adjacency, tab0, 0)
    r1 = edge_recip(adjacency, tab1, 1)
    f0, v0 = fill_streams(tab0, np.asarray(features0, dtype=np.float32), r0, profile, meta["perm"][0], cfg)
    f1, v1 = fill_streams(tab1, np.asarray(features1, dtype=np.float32), r1, profile, meta["perm"][1], cfg)
    iota = np.broadcast_to(
        np.arange(BLK_V).astype(ml_dtypes.bfloat16), (BLK, BLK_V)
    ).copy()
    iotar = np.broadcast_to(
        (np.arange(oh_chunk * BLK_V) // oh_chunk).astype(ml_dtypes.bfloat16),
        (BLK, oh_chunk * BLK_V),
    ).copy()

    in_maps = [
        {
            "feat0": f0[c],
            "feat1": f1[c],
            "vin0": v0[c],
            "vin1": v1[c],
            "iota": iota,
            "iotar": iotar,
        }
        for c in range(NCORES)
    ]
    return in_maps, profile, meta


def assemble_output(core_outs, meta, cfg=_DEFAULT_CFG):
    outs = []
    for t in range(T):
        perm = meta["perm"][t]
        vblk, vvin = meta["vmap"][t]
        parts = []
        for c in range(NCORES):
            arr = np.asarray(core_outs[c]).astype(np.float32)
            arr = arr.reshape(NGRP, BLK_V, G, T, F)[:, :, :, t, :]
            arr = arr.transpose(0, 2, 1, 3).reshape(NGRP * G, BLK_V, F)[:NBLK]
            tmp = np.empty((NBLK, BLK_V, F), dtype=np.float32)
            tmp[perm[c]] = arr
            parts.append(tmp[vblk[c, :VLOC], vvin[c, :VLOC]])
        outs.append(np.concatenate(parts, axis=0).reshape(B, V, F))
    return (outs[0], outs[1])


def kernel(adjacency, indices0, features0, indices1, features1):
    from concourse.bass_utils import run_bass_kernel_spmd

    cfg = _DEFAULT_CFG
    in_maps, profile, meta = prepare_inputs(
        adjacency, indices0, features0, indices1, features1, cfg
    )

    key = tuple(profile)
    if key not in _NC_CACHE:
        _NC_CACHE[key] = build_device_program(profile, cfg)
    nc = _NC_CACHE[key]

    res = run_bass_kernel_spmd(nc, in_maps, list(range(NCORES)))
    return assemble_output(
        [res.results[c]["out"] for c in range(NCORES)], meta, cfg
    )


# revision 31
# speedup vs baseline: 1.0478x; 1.0478x over previous
"""Trainium2 Bass kernel for GNN mean aggregation (nn_AggrGSMean).

Computes, for t in {0,1}:
    out_t[b, v, :] = segment_sum(features_t over edges with dest v) / degree[b, v, t]
where degree[b, v, t] = max(count(adjacency[b, v, t, :] >= 0), 1).

Strategy (graph-partition sharding per the problem's sharding hint):
- Host: partition edges by destination-vertex range across 8 cores, sort each
  core's edges by destination, and fold 1/degree into the features (f32
  multiply, then bf16).  Same-destination edges are PAIRED (Q=2, odd counts
  padded with a zero edge); a pair-row carries both edges' features in (q f)
  word order.  Vertices are bin-packed into 128-vertex blocks whose pair
  counts just fit 8 or 9 full 128-row tiles (serpentine deal by degree +
  swap repair), so the shared static per-slot profile (max over cores and
  tables at each sorted rank) wastes <2%.  Adjacent slots interleave per
  partition so one DMA loads a slot pair.  Destination slot-vertex ids ship
  as a separate bf16 "vin" stream [128, total_tiles].
- Device (per core): per slot-pair and table: one feature DMA (alternating
  the SP / Activation / GPSIMD DMA rings for aggregate bandwidth); one-hots
  [128 pair-rows x 128 vslots] built on DVE by iota==vin in a transposed
  (v i) all-packed layout batched 8 tiles per instruction (odd 9th tiles go
  to ScalarE via relu(1-(iota-v)^2) so DVE chunks keep even lhsT strides);
  per tile one matmul accumulates onehot.T @ pairs into a quarter of a
  shared PSUM tile [128, 4*(q f)] f32 (one full bank per slot-quad); one DVE
  reduce per same-group slot run adds the q halves straight into the bf16
  group output tile, DMA'd out per 7-slot group.
"""

import sys

if "/opt/trn_rl_repo" not in sys.path:
    sys.path.insert(0, "/opt/trn_rl_repo")

import ml_dtypes
import numpy as np

# Problem constants (hardcoded per contract)
B, V, T, N, F, M = 1, 100000, 2, 32, 64, 1600000
NCORES = 8
BLK = 128           # pair-rows per tile (matmul contraction)
BLK_V = 128         # vertices per block / one-hot width
Q = 2               # edges pre-summed per pair-row
ROW_W = Q * F       # bf16 words per pair-row (128)
G = 7               # slots per output group
VLOC = V // NCORES          # 12500
NBLK = -(-VLOC // BLK_V)    # 98
NGRP = -(-NBLK // G)        # 14
VPAD = NBLK * BLK_V         # 12544


class Cfg:
    def __init__(self):
        self.V = V
        self.NCORES = NCORES
        self.VLOC = VLOC
        self.NBLK = NBLK
        self.VPAD = VPAD


_DEFAULT_CFG = Cfg()
_NC_CACHE = {}


def build_device_program(
    profile, cfg=_DEFAULT_CFG, act_frac=0.06, oh_chunk=8, gp_mod=8, act_period=13
):
    """Build + compile the per-core Bass program.

    One-hot builds run on DVE in the transposed (v i) all-packed layout,
    batched oh_chunk tiles per instruction (stride-8 lhsT stays 16B-aligned
    for LDWEIGHTS); the odd 9th tile of 9-tile slots goes to ScalarE via
    relu(1-(iota-v)^2).  Feature DMAs alternate the SP and Activation HWDGE
    rings with every gp_mod'th on the GPSIMD SWDGE ring."""
    from contextlib import ExitStack

    import concourse.tile as tile
    from concourse import bacc, mybir

    f32 = mybir.dt.float32
    bf16 = mybir.dt.bfloat16
    assert len(profile) == NBLK
    prof = np.asarray(profile, dtype=np.int64)
    t_max = int(prof.max())
    TT = int(prof.sum())                      # total tiles per table
    # slot-PAIR interleaved stream: each partition's two slot segments are
    # contiguous, so one 2D DMA covers a pair of any widths
    wpair = (prof[0::2] + prof[1::2]) * ROW_W          # words per partition
    peb = np.concatenate([[0], np.cumsum(BLK * wpair)]).astype(np.int64)
    tb = np.concatenate([[0], np.cumsum(prof)]).astype(np.int64)
    total_elems = int(peb[-1])

    nc = bacc.Bacc("TRN2", target_bir_lowering=False, debug=False)
    feat_d = [
        nc.dram_tensor(f"feat{t}", [total_elems], bf16, kind="ExternalInput").ap()
        for t in range(T)
    ]
    vin_d = [
        nc.dram_tensor(f"vin{t}", [BLK, TT], bf16, kind="ExternalInput").ap()
        for t in range(T)
    ]
    iota_d = nc.dram_tensor("iota", [BLK, BLK_V], bf16, kind="ExternalInput").ap()
    iotar_d = nc.dram_tensor(
        "iotar", [BLK, oh_chunk * BLK_V], bf16, kind="ExternalInput"
    ).ap()
    out_d = nc.dram_tensor(
        "out", [NGRP, BLK_V, G * T * F], bf16, kind="ExternalOutput"
    ).ap()

    with tile.TileContext(nc) as tc, ExitStack() as ctx:
        const = ctx.enter_context(tc.tile_pool(name="const", bufs=1))
        featp = ctx.enter_context(tc.tile_pool(name="featp", bufs=8))
        ohdp = ctx.enter_context(tc.tile_pool(name="ohdp", bufs=6))
        ohap = ctx.enter_context(tc.tile_pool(name="ohap", bufs=7))
        outp = ctx.enter_context(tc.tile_pool(name="outp", bufs=3))
        psump = ctx.enter_context(tc.tile_pool(name="psum", bufs=8, space="PSUM"))

        # Const loads ride the Activation engine's HWDGE queues so they can
        # never get stuck behind feature DMAs (sync queues) that wait on
        # tile-pool recycling.
        iota_t = const.tile([BLK, BLK_V], bf16)
        nc.scalar.dma_start(out=iota_t[:], in_=iota_d[:])
        iotar_t = const.tile([BLK, oh_chunk * BLK_V], bf16)
        nc.scalar.dma_start(out=iotar_t[:], in_=iotar_d[:])
        vin_t = []
        vinf_t = []
        for t in range(T):
            vt = const.tile([BLK, TT], bf16, tag=f"vin{t}")
            # 4-chunk column split so the load spreads across DMA queues
            bnds = [TT * i // 4 for i in range(5)]
            for a, b in zip(bnds[:-1], bnds[1:]):
                if b > a:
                    nc.scalar.dma_start(out=vt[:, a:b], in_=vin_d[t][:, a:b])
            vin_t.append(vt)
            # f32 negated copy for the ScalarE activation-bias one-hot path
            vf = const.tile([BLK, TT], f32, tag=f"vinf{t}")
            nc.vector.tensor_scalar(
                vf[:], vt[:], -1.0, None, op0=mybir.AluOpType.mult
            )
            vinf_t.append(vf)

        # --- one-hot build planning: odd slots give their last tile to
        # ScalarE so every DVE chunk keeps an even lhsT stride (aligned
        # LDWEIGHTS); remaining tiles batch in chunks of oh_chunk.  Every
        # act_period'th slot-table goes entirely to ScalarE to offload DVE.
        slot_ctr = [0]

        def plan_chunks(t_s):
            slot_ctr[0] += 1
            chunks = []
            end = t_s
            if t_s % 2 == 1 and t_s > 1:
                chunks.append(("act", t_s - 1, 1))
                end = t_s - 1
            i = 0
            while i < end:
                k = min(oh_chunk, end - i)
                chunks.append(("b", i, k))
                i += k
            return chunks

        ndma = [0]
        out_tiles = {}

        def compute_slot(s, t, feat_t, off, ps, half):
            """One (slot, table) pass: one-hots + matmuls into psum half."""
            t_s = int(prof[s])
            g, so = divmod(s, G)
            if g not in out_tiles:
                ot = outp.tile([BLK_V, G * T * F], bf16, tag="outg")
                out_tiles[g] = ot
            refs = [None] * t_s
            for kind, i0, k in plan_chunks(t_s):
                c0 = int(tb[s]) + i0
                if kind == "act":
                    bias = vinf_t[t][:, c0 : c0 + 1]
                    y = ohap.tile([BLK, BLK_V], bf16, tag="y")
                    nc.scalar.activation(
                        y[:], iota_t[:],
                        mybir.ActivationFunctionType.Square,
                        bias=bias, scale=1.0,
                    )
                    oh = ohap.tile([BLK, BLK_V], bf16, tag="oha")
                    nc.scalar.activation(
                        oh[:], y[:], mybir.ActivationFunctionType.Relu,
                        bias=1.0, scale=-1.0,
                    )
                    refs[i0] = (oh, 0, 1)
                else:
                    oh = ohdp.tile([BLK, oh_chunk * BLK_V], bf16, tag="ohb")
                    oh3 = oh[:, : k * BLK_V].rearrange("p (v i) -> p v i", i=k)
                    in0 = iotar_t[:].rearrange(
                        "p (v i) -> p v i", i=oh_chunk
                    )[:, :, :k]
                    in1 = (
                        vin_t[t][:, c0 : c0 + k]
                        .unsqueeze(1)
                        .broadcast_to([BLK, BLK_V, k])
                    )
                    nc.vector.tensor_tensor(
                        oh3, in0, in1, op=mybir.AluOpType.is_equal
                    )
                    for j in range(k):
                        refs[i0 + j] = (oh, j, k)
            for i in range(t_s):
                oh, idx, stride = refs[i]
                if stride == 1:
                    lhsT = oh[:, idx : idx + BLK_V]
                else:
                    lhsT = oh[:, : stride * BLK_V].rearrange(
                        "p (v i) -> p i v", i=stride
                    )[:, idx, :]
                nc.tensor.matmul(
                    ps[:, half * ROW_W : (half + 1) * ROW_W],
                    lhsT=lhsT,
                    rhs=feat_t[:, (off + i) * ROW_W : (off + i + 1) * ROW_W],
                    start=(i == 0),
                    stop=(i == t_s - 1),
                )

        for q0 in range(0, NBLK, 4):
            qn = min(4, NBLK - q0)
            for t in range(T):
                fts = []
                for pp in range(0, qn, 2):
                    pid = (q0 + pp) // 2
                    w2 = int(wpair[pid])
                    feat_t = featp.tile(
                        [BLK, 2 * t_max * ROW_W], bf16, tag="feat"
                    )
                    srcap = feat_d[t][
                        int(peb[pid]) : int(peb[pid + 1])
                    ].rearrange("(e w) -> e w", w=w2)
                    ndma[0] += 1
                    if ndma[0] % gp_mod == 0:
                        deng = nc.gpsimd
                    elif ndma[0] % 2 == 0:
                        deng = nc.scalar
                    else:
                        deng = nc.sync
                    deng.dma_start(out=feat_t[:, :w2], in_=srcap)
                    fts.append(feat_t)
                ps = psump.tile([BLK_V, 4 * ROW_W], f32)
                for pp in range(0, qn, 2):
                    s0 = q0 + pp
                    compute_slot(s0, t, fts[pp // 2], 0, ps, pp)
                    compute_slot(
                        s0 + 1, t, fts[pp // 2], int(prof[s0]), ps, pp + 1
                    )
                # reduce per same-group slot run within the quad
                with nc.allow_low_precision(reason="bf16 mean output"):
                    s = q0
                    while s < q0 + qn:
                        g = s // G
                        e = min(q0 + qn, (g + 1) * G)
                        L = e - s
                        o = s - q0
                        so = s % G
                        vv = out_tiles[g][:].rearrange(
                            "p (so tt f) -> p so tt f", so=G, tt=T
                        )
                        nc.vector.tensor_reduce(
                            vv[:, so : so + L, t, :],
                            ps[
                                :, o * ROW_W : (o + L) * ROW_W
                            ].rearrange(
                                "p (sL q f) -> p sL f q", sL=L, q=Q
                            ),
                            axis=mybir.AxisListType.X,
                            op=mybir.AluOpType.add,
                        )
                        s = e
            for s in range(q0, q0 + qn):
                if s % G == G - 1:
                    g = s // G
                    nc.sync.dma_start(out=out_d[g], in_=out_tiles.pop(g)[:])

    nc.compile()
    return nc


def _pack_core(pv, x_high):
    """Pack VPAD vertices (pair counts pv) into NBLK bins of exactly BLK_V.

    Serpentine-deal by descending pv (near-equal sums), then swap-repair so
    at most x_high bins exceed 8 tiles (cap 9).  Returns bins [NBLK, BLK_V]
    of vertex ids."""
    order = np.argsort(-pv, kind="stable")
    deal = order.reshape(BLK_V, NBLK).copy()
    deal[1::2] = deal[1::2, ::-1]
    bins = np.ascontiguousarray(deal.T)          # [NBLK, BLK_V]
    sums = pv[bins].sum(axis=1)
    lo_cap, hi_cap = 8 * BLK, 9 * BLK
    hi = set(np.argsort(-sums)[:x_high])
    room = {h: hi_cap - sums[h] for h in hi}
    for b in range(NBLK):
        if b in hi:
            continue
        need = sums[b] - lo_cap
        it = 0
        while need > 0 and it < 40:
            it += 1
            h = max(room, key=room.get)
            if room[h] <= 0:
                break
            pb_ = pv[bins[b]]
            ph_ = pv[bins[h]]
            iu = int(np.argmax(pb_))
            iw = int(np.argmin(ph_))
            gain = int(pb_[iu] - ph_[iw])
            if gain <= 0:
                break
            gain = min(gain, int(room[h]) + 0)
            u, w = bins[b, iu], bins[h, iw]
            if pv[u] - pv[w] > room[h]:
                # find a better-matched u: largest pv[u] with delta <= room
                cand = np.where(pb_ - pv[w] <= room[h])[0]
                if len(cand) == 0:
                    break
                iu = int(cand[np.argmax(pb_[cand])])
                u = bins[b, iu]
                gain = int(pv[u] - pv[w])
                if gain <= 0:
                    break
            bins[b, iu], bins[h, iw] = w, u
            sums[b] -= gain
            sums[h] += gain
            room[h] -= gain
            need -= gain
    return bins


def shard_table(indices, x_high=None, cfg=_DEFAULT_CFG):
    """Per-edge placement: sort by destination, pair same-dest edges, and
    bin-pack vertices into blocks so slot tile counts are (mostly) 8 or 9."""
    v = np.ascontiguousarray(indices[:, 1]).astype(np.int64)
    order = np.argsort(v, kind="stable")
    vs = v[order]
    n_v = np.bincount(vs, minlength=V)
    starts = np.concatenate([[0], np.cumsum(n_v)])
    r = np.arange(len(vs), dtype=np.int64) - starts[vs]
    pv = (n_v + 1) // 2                                  # pairs per vertex
    core = vs // VLOC
    vloc_e = vs % VLOC

    pv_pad = np.zeros((NCORES, VPAD), dtype=np.int64)
    pv_pad[:, :VLOC] = pv.reshape(NCORES, VLOC)
    if x_high is None:
        tp = pv_pad.sum(axis=1)
        x_high = int(np.ceil((tp.max() - NBLK * 8 * BLK) / BLK)) + 1
        x_high = max(0, min(NBLK, x_high))

    vblk = np.empty((NCORES, VPAD), dtype=np.int64)
    vvin = np.empty((NCORES, VPAD), dtype=np.int64)
    pbb = np.empty((NCORES, VPAD), dtype=np.int64)   # pair base within block
    cnt = np.empty((NCORES, NBLK), dtype=np.int64)
    for c in range(NCORES):
        bins = _pack_core(pv_pad[c], x_high)
        flat = bins.reshape(-1)                      # slot index -> vertex
        slot_of = np.empty(VPAD, dtype=np.int64)
        slot_of[flat] = np.arange(VPAD)
        vblk[c] = slot_of // BLK_V
        vvin[c] = slot_of % BLK_V
        pv_by_slot = pv_pad[c][flat]
        cum = np.cumsum(pv_by_slot) - pv_by_slot     # exclusive
        blk_start = cum.reshape(NBLK, BLK_V)[:, 0]
        pb_by_slot = cum - np.repeat(blk_start, BLK_V)
        pbb[c] = pb_by_slot[slot_of]
        cnt[c] = pv_by_slot.reshape(NBLK, BLK_V).sum(axis=1)

    blk_e = vblk[core, vloc_e]
    vin_e = vvin[core, vloc_e]
    pr = pbb[core, vloc_e] + r // 2
    tiles = np.maximum(-(-cnt // BLK), 1)            # [NCORES, NBLK]
    return {
        "order": order, "core": core, "blk": blk_e, "vin": vin_e,
        "pr": pr, "q": (r & 1).astype(np.int64), "tiles": tiles,
        "vglob": vs, "vblk": vblk, "vvin": vvin,
    }


def make_profile(tables):
    """Shared slot tile profile + per (table, core) slot->block permutation.

    Returns (profile, meta) where meta carries the permutations and the
    per-table vertex->(block, vin) maps needed for output assembly."""
    perms = []
    sorted_tiles = []
    for tab in tables:
        perms_t = []
        for c in range(NCORES):
            tl = tab["tiles"][c]
            p = np.argsort(-tl, kind="stable")
            perms_t.append(p)
            sorted_tiles.append(tl[p])
        perms.append(np.stack(perms_t))
    profile = np.max(np.stack(sorted_tiles), axis=0)
    profile = np.maximum(profile, 1)
    meta = {
        "perm": perms,
        "vmap": [(tab["vblk"], tab["vvin"]) for tab in tables],
    }
    return [int(x) for x in profile], meta


def fill_streams(tab, features, rec_e, profile, perm, cfg=_DEFAULT_CFG):
    """Per-core bf16 feature stream (pre-scaled by 1/degree) + vin stream.

    Row layout is (q f): word q*F + f, so the PSUM pair halves are the
    contiguous column blocks [0:F] and [F:2F]."""
    prof = np.asarray(profile, dtype=np.int64)
    TT = int(prof.sum())
    wpair = (prof[0::2] + prof[1::2]) * ROW_W
    peb = np.concatenate([[0], np.cumsum(BLK * wpair)]).astype(np.int64)
    tb = np.concatenate([[0], np.cumsum(prof)]).astype(np.int64)
    TW = int(peb[-1])
    soff = np.zeros(NBLK, dtype=np.int64)
    soff[1::2] = prof[0::2] * ROW_W

    inv = np.empty((NCORES, NBLK), dtype=np.int64)
    for c in range(NCORES):
        inv[c, perm[c]] = np.arange(NBLK)

    scaled = features[tab["order"]] * rec_e[:, None]
    hi_u = scaled.astype(ml_dtypes.bfloat16).view(np.uint16)

    core = tab["core"]
    s_e = inv[core, tab["blk"]]
    p = tab["pr"] & (BLK - 1)
    i = tab["pr"] >> 7
    q = tab["q"]
    pid_e = s_e >> 1
    pos = (
        core * TW + peb[pid_e] + p * wpair[pid_e] + soff[s_e] + i * ROW_W + q * F
    )
    stream = np.zeros(NCORES * TW, dtype=np.uint16)
    cols = np.arange(F, dtype=np.int64)[None, :]
    stream[pos[:, None] + cols] = hi_u
    stream = stream.reshape(NCORES, TW).view(ml_dtypes.bfloat16)

    # vin stream [NCORES, 128, TT]; padding rows get -1 (never matches iota)
    vin_arr = np.full(NCORES * BLK * TT, -1.0, dtype=ml_dtypes.bfloat16)
    m0 = q == 0
    flat = core[m0] * (BLK * TT) + p[m0] * TT + (tb[s_e[m0]] + i[m0])
    vin_arr[flat] = tab["vin"][m0].astype(ml_dtypes.bfloat16)
    vin_arr = vin_arr.reshape(NCORES, BLK, TT)
    return stream, vin_arr


def edge_recip(adjacency, tab, t):
    """1/degree at each sorted edge's destination for table t."""
    adj = np.asarray(adjacency).reshape(V, T, N)
    deg = np.maximum((adj[:, t] >= 0).sum(axis=-1), 1).astype(np.float64)  # [V]
    rec = (1.0 / deg).astype(np.float32)
    return rec[tab["vglob"]]


def prepare_inputs(adjacency, indices0, features0, indices1, features1, cfg=_DEFAULT_CFG, oh_chunk=8):
    tab0 = shard_table(np.asarray(indices0), cfg=cfg)
    tab1 = shard_table(np.asarray(indices1), cfg=cfg)
    profile, meta = make_profile([tab0, tab1])

    r0 = edge_recip(adjacency, tab0, 0)
    r1 = edge_recip(adjacency, tab1, 1)
    f0, v0 = fill_streams(tab0, np.asarray(features0, dtype=np.float32), r0, profile, meta["perm"][0], cfg)
    f1, v1 = fill_streams(tab1, np.asarray(features1, dtype=np.float32), r1, profile, meta["perm"][1], cfg)
    iota = np.broadcast_to(
        np.arange(BLK_V).astype(ml_dtypes.bfloat16), (BLK, BLK_V)
    ).copy()
    iotar = np.broadcast_to(
        (np.arange(oh_chunk * BLK_V) // oh_chunk).astype(ml_dtypes.bfloat16),
        (BLK, oh_chunk * BLK_V),
    ).copy()

    in_maps = [
        {
            "feat0": f0[c],
            "feat1": f1[c],
            "vin0": v0[c],
            "vin1": v1[c],
            "iota": iota,
            "iotar": iotar,
        }
        for c in range(NCORES)
    ]
    return in_maps, profile, meta


def assemble_output(core_outs, meta, cfg=_DEFAULT_CFG):
    outs = []
    for t in range(T):
        perm = meta["perm"][t]
        vblk, vvin = meta["vmap"][t]
        parts = []
        for c in range(NCORES):
            arr = np.asarray(core_outs[c]).astype(np.float32)
            arr = arr.reshape(NGRP, BLK_V, G, T, F)[:, :, :, t, :]
            arr = arr.transpose(0, 2, 1, 3).reshape(NGRP * G, BLK_V, F)[:NBLK]
            tmp = np.empty((NBLK, BLK_V, F), dtype=np.float32)
            tmp[perm[c]] = arr
            parts.append(tmp[vblk[c, :VLOC], vvin[c, :VLOC]])
        outs.append(np.concatenate(parts, axis=0).reshape(B, V, F))
    return (outs[0], outs[1])


def kernel(adjacency, indices0, features0, indices1, features1):
    from concourse.bass_utils import run_bass_kernel_spmd

    cfg = _DEFAULT_CFG
    in_maps, profile, meta = prepare_inputs(
        adjacency, indices0, features0, indices1, features1, cfg
    )

    key = tuple(profile)
    if key not in _NC_CACHE:
        _NC_CACHE[key] = build_device_program(profile, cfg)
    nc = _NC_CACHE[key]

    res = run_bass_kernel_spmd(nc, in_maps, list(range(NCORES)))
    return assemble_output(
        [res.results[c]["out"] for c in range(NCORES)], meta, cfg
    )
